# revision 74
# baseline (speedup 1.0000x reference)
"""Trainium2 Bass kernel for LocalAttention: sliding-window attention gate +
per-position linear + tanh + global maxpool.

out[b,c] = tanh(max_l( sigmoid(conv1d(x, W_att) + b_att)[l] * (W_cnn @ x[b].T)[c,l] ) + b_cnn[c])

Sharding: data-parallel over batch B=64 across 8 cores (8 batches/core).

v2: fp8 (e4m3) DoubleRow matmuls with 3-term error compensation
  u = W_hi*x_hi + W_hi*x_lo + W_lo*x_hi   (x_hi/x_lo, W_hi/W_lo e4m3 splits)
packed as DoubleRow (p, j) contraction pairs (2 slots per matmul, 0.5 cyc/col)
with a shared x_hi/x_lo-interleaved rhs. Per-core pipeline, 3-slot score skew:
  front(b): chunk1 DR-matmuls -> ACT evac bf16 -> att rows to DRAM scratch
  back(b):  diagonal scratch read -> ones-matmul broadcast -> ACT sigmoid
            (scale=1/S folds the fp8 descale) -> JIT chunk0 DR-matmuls ->
            DVE gate-from-psum + pairwise max + reduce (chunk1 gate on GpSimd)
x loads stream as ec-quarter DMAs so the serial DMA queue stays shallow and
scratch hops interleave; consts issue on the ACT HWDGE path; final tanh folds
b_cnn and the 1/S descale.
"""

import functools
import sys

import ml_dtypes
import numpy as np

sys.path.insert(0, "/opt/trn_rl_repo")

import concourse.bacc as bacc
import concourse.bass as bass
import concourse.tile as tile
from concourse import mybir
from concourse.bass_utils import run_bass_kernel_spmd

B, L, E, WIN, C = 64, 1024, 512, 5, 200
NCORES = 8
BS = B // NCORES  # batches per core
P = 128
EC = E // P       # 4 contraction chunks of 128
ECP = EC // 2     # 2 DoubleRow ec-pairs
LH = L // 2       # 512, one PSUM bank
# augmented output channels: 200 cnn + zero pad + 5 att rows at UOFF of chunk 1
UOFF = 96         # partition offset of W_att rows inside c-chunk 1
CAUG = 240        # aug rows padded so the DR lhsT Ko-step (CAUG) is %16==0
CCH = [(0, P), (P, UOFF + WIN)]   # (start, rows) of the two matmul chunks
CW1 = C - P                       # valid cnn rows in chunk 1 (72)
SROW = L + 4      # DRAM scratch row length (2-col zero pad)
SKEW = 3          # slots between chunk1/score launch and gate/reduce
NFAST = 0         # trailing batches whose scores skip DRAM (PE shifted mms)
S = 64.0          # weight scaling for fp8

FP32 = mybir.dt.float32
BF16 = mybir.dt.bfloat16
FP8 = mybir.dt.float8e4
AF = mybir.ActivationFunctionType
ALU = mybir.AluOpType
DR = mybir.MatmulPerfMode.DoubleRow
NEG = -3.0e38


def _body(nc, tc, x_d, wh_d, wl_d, batt_d, bcnn_d, out_d):
    with (
        tc.tile_pool(name="const", bufs=1) as cpool,
        tc.tile_pool(name="xin", bufs=BS) as xpool,
        tc.tile_pool(name="vg1", bufs=SKEW + 2) as vg1pool,
        tc.tile_pool(name="ssb", bufs=3) as spool,
        tc.tile_pool(name="tr", bufs=3) as trpool,
        tc.tile_pool(name="u", bufs=4) as upool,
        tc.tile_pool(name="oacc", bufs=1) as opool,
        tc.tile_pool(name="dsc", bufs=1, space="DRAM") as dpool,
        tc.tile_pool(name="pv", bufs=3, space="PSUM") as pvpool,
        tc.tile_pool(name="ps", bufs=1, space="PSUM") as pspool,
    ):
        # ---- prologue: small consts FIRST (so the serial DMA device isn't
        # holding sall/batt hostage behind multi-us x loads), then weights,
        # then x loads with PE warmup overlapped. ----
        sall = dpool.tile([BS, WIN, SROW], BF16, tag="sall")
        zed = cpool.tile([WIN, 4 * BS], BF16, tag="zed")
        nc.gpsimd.memset(zed[:], 0.0)
        ones_sb = cpool.tile([WIN, P], BF16, tag="ones")
        nc.gpsimd.memset(ones_sb[:], 1.0)

        # preload the Tanh+Sigmoid activation tables off the critical path
        warm_t = cpool.tile([1, 2], FP32, tag="warmt")
        nc.scalar.activation(out=warm_t[:, 0:1], in_=zed[0:1, 0:1], func=AF.Tanh)
        nc.scalar.activation(out=warm_t[:, 1:2], in_=zed[0:1, 0:1], func=AF.Sigmoid)

        # consts on the ACT-issued DMA path; x/weights own the SP issue pipe
        batt_sb = cpool.tile([P, 1], FP32, tag="batt")
        nc.scalar.dma_start(out=batt_sb[:], in_=batt_d)
        bcnn_sb = []
        for ci, (c0, cw) in enumerate([(0, P), (P, CW1)]):
            t = cpool.tile([cw, 1], FP32, tag=f"bcnn{ci}")
            nc.scalar.dma_start(out=t[:], in_=bcnn_d[c0 : c0 + cw, :])
            bcnn_sb.append(t)
        sbase = sall[:]
        for edge_off in (0, L + 2):
            nc.scalar.dma_start(
                out=bass.AP(
                    sbase.tensor,
                    sbase.offset + edge_off,
                    [[SROW, WIN], [WIN * SROW, BS], [1, 2]],
                ),
                in_=zed[:, 0 : 2 * BS].rearrange("p (b c) -> p b c", c=2),
            )

        whl_sb = cpool.tile([P, 2, ECP, 2, CAUG], FP8, tag="whl")
        wh_sb = whl_sb[:, 0]
        wl_sb = whl_sb[:, 1]
        nc.sync.dma_start(
            out=whl_sb[:], in_=wh_d.rearrange("hl ep p j c -> p hl ep j c")
        )

        xts = {}

        def xload(b):
            # ec-quarter pieces: transfer time ~= issue time, so the serial
            # DMA device queue stays shallow and scratch hops interleave.
            xts[b] = xpool.tile([P, EC, 2, L], FP8, tag="xt", name=f"xt{b}")
            for ec in range(EC):
                nc.sync.dma_start(out=xts[b][:, ec], in_=x_d[b, ec])

        xload(0)
        for b in range(1, min(SKEW + 1, BS)):
            xload(b)

        # warmup matmuls to ramp the PE pstate during initial DMAs
        wpv = pspool.tile([P, 2, LH], FP32, tag="ps", name="wpv")
        for _ in range(10):
            nc.tensor.matmul(
                wpv[:, 0, 0:CAUG],
                lhsT=wh_sb[:, 0, :, 0:P],
                rhs=wh_sb[:, 0, :, :],
                perf_mode=DR,
                start=True,
                stop=True,
            )

        oacc0 = opool.tile([P, BS], FP32, tag="oacc0")
        oacc1 = opool.tile([CW1, BS], FP32, tag="oacc1")
        vg1s = {}

        def mm_chunk(b, ci, pv):
            """6 DR matmuls per (chunk, lt): A=Wh*xhi, B=Wh*xlo, C=Wl*xhi."""
            c0, cw = CCH[ci]
            xT = xts[b]
            for lt in range(2):
                k = 0
                for wsb, ioff in ((wh_sb, 0), (wh_sb, 1), (wl_sb, 0)):
                    for ep in range(ECP):
                        # rhs: x slots (ec=2ep+j, i=ioff) for j in 0..1
                        nc.tensor.matmul(
                            pv[:cw, lt, :],
                            lhsT=wsb[:, ep, :, c0 : c0 + cw],
                            rhs=xT[:, 2 * ep : 2 * ep + 2, ioff, lt * LH : (lt + 1) * LH],
                            perf_mode=DR,
                            start=(k == 0),
                            stop=(k == 5),
                        )
                        k += 1

        def stage_front(b):
            """chunk1 matmul -> ACT evac (bf16) -> att rows to DRAM scratch."""
            pv1 = pvpool.tile([UOFF + WIN, 2, LH], FP32, tag="pv", name=f"pv1_{b}")
            mm_chunk(b, 1, pv1)
            vg1 = vg1pool.tile([UOFF + WIN, L + 4], BF16, tag="vg1", name=f"vg1_{b}")
            nc.scalar.copy(
                out=vg1[:, 2 : L + 2],
                in_=pv1[:, :, :].rearrange("c a b -> c (a b)"),
            )
            vg1s[b] = vg1
            if b >= BS - NFAST:
                # fast-path batch: zero the pad edges of the att rows; scores
                # will be computed on-PE from this tile (no DRAM round trip)
                nc.gpsimd.memset(vg1[UOFF : UOFF + WIN, 0:2], 0.0)
                nc.gpsimd.memset(vg1[UOFF : UOFF + WIN, L + 2 : L + 4], 0.0)
            else:
                nc.scalar.dma_start(
                    out=sall[b, :, 2 : L + 2],
                    in_=vg1[UOFF : UOFF + WIN, 2 : L + 2],
                )

        ualis = {}

        def scratch_in(b):
            uali = upool.tile([WIN, L], BF16, tag="uali", name=f"uali{b}")
            nc.scalar.dma_start(
                out=uali[:],
                in_=bass.AP(
                    sbase.tensor,
                    sbase.offset + b * WIN * SROW,
                    [[SROW + 1, WIN], [1, L]],
                ),
            )
            ualis[b] = uali

        def stage_back(b):
            """ones-mm -> sigmoid -> chunk0 mm -> fused TTRs."""
            ps = pspool.tile([P, 2, LH], FP32, tag="ps", name=f"ps{b}")
            if b >= BS - NFAST:
                uali = None
                vg1p = vg1s[b]
                for lt in range(2):
                    for w in range(WIN):
                        nc.tensor.matmul(
                            ps[:, lt, :],
                            lhsT=sel_sb[UOFF : UOFF + WIN, w, :],
                            rhs=vg1p[
                                UOFF : UOFF + WIN,
                                lt * LH + w : lt * LH + w + LH,
                            ],
                            start=(w == 0),
                            stop=(w == WIN - 1),
                            tile_position=(UOFF, 0),
                        )
            else:
                uali = ualis.pop(b)
                for lt in range(2):
                    nc.tensor.matmul(
                        ps[:, lt, :],
                        lhsT=ones_sb[:],
                        rhs=uali[:, lt * LH : (lt + 1) * LH],
                        start=True,
                        stop=True,
                    )
            ssb = spool.tile([P, L], BF16, tag="ssb", name=f"ssb{b}")
            nc.scalar.activation(
                out=ssb[:],
                in_=ps[:, :, :].rearrange("c a b -> c (a b)"),
                func=AF.Sigmoid,
                bias=batt_sb[:],
                scale=1.0 / S,
            )
            # value-preserving touch of xt(b)'s corner, gated on the score
            # arriving: keeps the scheduler from hoisting chunk0's matmuls,
            # which would pin pv0 psum slots long before the TTR can drain.
            if False and uali is not None:
                nc.vector.tensor_tensor(
                    out=xts[b][0:1, 0, 0, 0:1],
                    in0=xts[b][0:1, 0, 0, 0:1],
                    in1=uali[0:1, 0:1],
                    op=ALU.bypass,
                )
            pv0 = pvpool.tile([P, 2, LH], FP32, tag="pv", name=f"pv0_{b}")
            mm_chunk(b, 0, pv0)
            vg1 = vg1s.pop(b)
            g0 = trpool.tile([P, L], BF16, tag="tr0")
            nc.vector.tensor_tensor(
                out=g0[:],
                in0=pv0[:, :, :].rearrange("c a b -> c (a b)"),
                in1=ssb[:],
                op=ALU.mult,
            )
            g1 = trpool.tile([CW1, L], BF16, tag="tr1")
            nc.gpsimd.tensor_tensor(
                out=g1[:], in0=vg1[:CW1, 2 : L + 2], in1=ssb[:CW1, :], op=ALU.mult
            )
            h0 = trpool.tile([P, LH], BF16, tag="h0")
            nc.vector.tensor_tensor(
                out=h0[:], in0=g0[:, 0:LH], in1=g0[:, LH:L], op=ALU.max
            )
            nc.vector.reduce_max(
                oacc0[:, b : b + 1], h0[:], axis=mybir.AxisListType.X
            )
            h1 = trpool.tile([CW1, LH], BF16, tag="h1")
            nc.vector.tensor_tensor(
                out=h1[:], in0=g1[:, 0:LH], in1=g1[:, LH:L], op=ALU.max
            )
            nc.vector.reduce_max(
                oacc1[:, b : b + 1], h1[:], axis=mybir.AxisListType.X
            )

        # explicit pipeline timeline (lower bounds, ns): fronts paced by x
        # arrival, scratch-in after the out-hop lands, backs after the round
        # trip. Keeps the tile scheduler from serializing back behind front.
        SLOT = 2950.0
        FILL = 5300.0

        def F(t):
            return (FILL + SLOT * t) / 1e6

        SKIN = 2  # slots between scratch-out and scratch-in issue
        for t in range(BS + SKEW):
            if t + SKEW + 1 < BS:
                xload(t + SKEW + 1)
            if SKIN <= t < BS - NFAST + SKIN:
                scratch_in(t - SKIN)
            if t >= SKEW:
                stage_back(t - SKEW)
            if t < BS:
                stage_front(t)
            if t >= SKEW:
                xts.pop(t - SKEW)

        # ---- tanh(max/S + b_cnn) and store ----
        for ci, (c0, cw, acc) in enumerate([(0, P, oacc0), (P, CW1, oacc1)]):
            of = spool.tile([cw, BS], FP32, tag=f"of{ci}")
            nc.scalar.activation(
                out=of[:], in_=acc[:], func=AF.Tanh, bias=bcnn_sb[ci][:],
                scale=1.0 / S,
            )
            nc.sync.dma_start(out=out_d[c0 : c0 + cw, :], in_=of[:])


@functools.lru_cache(maxsize=1)
def _build():
    nc = bacc.Bacc(
        "TRN2",
        target_bir_lowering=False,
        debug=False,
        enable_asserts=False,
        num_devices=NCORES,
    )
    x_d = nc.dram_tensor("xT", [BS, EC, P, 2, L], FP8, kind="ExternalInput").ap()
    wh_d = nc.dram_tensor(
        "whlT", [2, ECP, P, 2, CAUG], FP8, kind="ExternalInput"
    ).ap()
    wl_d = None
    batt_d = nc.dram_tensor("b_att_b", [P, 1], FP32, kind="ExternalInput").ap()
    bcnn_d = nc.dram_tensor("b_cnn_c", [2 * P, 1], FP32, kind="ExternalInput").ap()
    out_d = nc.dram_tensor("out", [C, BS], FP32, kind="ExternalOutput").ap()
    with tile.TileContext(nc) as tc:
        _body(nc, tc, x_d, wh_d, wl_d, batt_d, bcnn_d, out_d)
    nc.compile()
    return nc


def _prep_in_maps(x, W_att, b_att, W_cnn, b_cnn):
    e4 = ml_dtypes.float8_e4m3
    pad1 = np.zeros((P + UOFF - C, E), dtype=np.float32)
    pad2 = np.zeros((CAUG - P - UOFF - WIN, E), dtype=np.float32)
    waug = (np.concatenate([W_cnn, pad1, W_att, pad2], axis=0) * S).astype(
        np.float32
    )
    wh = waug.astype(e4)
    wl = (waug - wh.astype(np.float32)).astype(e4)
    # lhsT layout [ECP, P, 2, CAUG]: w[ep, p, j, c] = W[c, (2*ep + j)*P + p]
    def wlayout(w8):
        wt = np.ascontiguousarray(w8.T)              # [E, CAUG]
        return wt.reshape(ECP, 2, P, CAUG).transpose(0, 2, 1, 3).copy()
    whlT = np.stack([wlayout(wh), wlayout(wl)], axis=0)
    batt = np.full((P, 1), np.float32(b_att[0]), dtype=np.float32)
    bcnn = np.zeros((2 * P, 1), dtype=np.float32)
    bcnn[:C, 0] = np.asarray(b_cnn, dtype=np.float32)
    # x: [B, L, E] -> hi/lo e4m3 splits in layout [B, EC, P, 2, L]
    xf = np.asarray(x, dtype=np.float32)
    xT = np.ascontiguousarray(xf.transpose(0, 2, 1))  # [B, E, L]
    xh = xT.astype(e4)
    xl = (xT - xh.astype(np.float32)).astype(e4)
    xhl = np.stack([xh, xl], axis=2)                  # [B, E, 2, L]
    xhl = xhl.reshape(B, EC, P, 2, L)
    in_maps = []
    for c in range(NCORES):
        in_maps.append(
            {
                "xT": xhl[c * BS : (c + 1) * BS],
                "whlT": whlT,
                "b_att_b": batt,
                "b_cnn_c": bcnn,
            }
        )
    return in_maps


def run(x, W_att, b_att, W_cnn, b_cnn, trace=False):
    nc = _build()
    in_maps = _prep_in_maps(x, W_att, b_att, W_cnn, b_cnn)
    res = run_bass_kernel_spmd(nc, in_maps, core_ids=list(range(NCORES)), trace=trace)
    outs = [r["out"] for r in res.results]  # each [C, BS]
    out = np.concatenate([o.T for o in outs], axis=0)  # [B, C]
    return out[:, :, None, None].astype(np.float32), res


def kernel(x, W_att, b_att, W_cnn, b_cnn):
    out, _ = run(x, W_att, b_att, W_cnn, b_cnn)
    return out
